# revision 21
# baseline (speedup 1.0000x reference)
"""CELPNet Trainium2 kernel: cond-net + 800-step autoregressive GRU scan.

Strategy:
- 8 cores, data-parallel over batch (64/core); per core, 2 interleaved
  batch-32 shards with fully separate SBUF/PSUM pools so their serial
  gate chains overlap on the engines.
- All matmuls weight-stationary (lhsT = W packed [in,out], activations
  as the moving rhs in feature-major [feat_part, batch_free] layout) ->
  no transposes anywhere.
- bf16 matmul operands, fp32 PSUM accumulation, fp32 carried state and
  gate arithmetic (measured end-to-end drift vs fp32: rel l2 ~7.5e-3,
  vs the typical 2e-2 gate).
- d1's cond contribution is precomputed per frame outside the scan (cond
  repeats 4x per frame -> only 200 unique frames, ~13MB resident SBUF),
  leaving only the tiny prev@Wp matmul (K=40) in the loop.
- PSUM has_written is cleared bank-wide by start=True, so only the
  chronologically first matmul into each bank per generation carries it.
- whh (gh-side) matmuls for all 3 GRUs are issued at step start (they
  depend only on h(s-1)) so the PE prefills PSUM while gates evaluate;
  u=z*h and w=1-z run on gpsimd off the critical path.
"""
import sys

sys.path.insert(0, "/opt/trn_rl_repo")

import numpy as np
import ml_dtypes
from contextlib import ExitStack

import concourse.bass as bass
import concourse.tile as tile
import concourse.mybir as mybir
from concourse import bacc
from concourse.bass_utils import run_bass_kernel_spmd

BF16 = mybir.dt.bfloat16
F32 = mybir.dt.float32
AF = mybir.ActivationFunctionType
ALU = mybir.AluOpType

NCORES = 8
B = 512
T = 204
FEAT = 20
C = 256
SUB = 40
NB = 200          # frames
NSUB = 4
S = NB * NSUB     # 800 steps
SHARDS = 2
BS = 64 // SHARDS   # batch lanes per shard
BQ = 16             # quarter-of-core batch for cond-net staging
TBQ = T * BQ        # featT cols per quarter


def _ceil_div(a, b):
    return (a + b - 1) // b


def build_nc(nb=NB, shards=SHARDS, trace_label="", debug_d1c=False):
    """Build the Bass program (same program runs SPMD on all 8 cores)."""
    s_total = nb * NSUB
    nc = bacc.Bacc(
        "TRN2", target_bir_lowering=False, debug=False,
        enable_asserts=False, num_devices=NCORES,
    )

    # ---- DRAM params (per-core shards / replicated weights) ----
    featT = nc.declare_dram_parameter("featT", [FEAT, 4 * TBQ], BF16, isOutput=False)
    w_fd1 = nc.declare_dram_parameter("w_fd1", [FEAT, C], BF16, isOutput=False)
    w_c1 = nc.declare_dram_parameter("w_c1", [128, 3 * 2 * C], BF16, isOutput=False)
    w_c2 = nc.declare_dram_parameter("w_c2", [128, 3 * 2 * C], BF16, isOutput=False)
    w_fd2 = nc.declare_dram_parameter("w_fd2", [128, 2 * C], BF16, isOutput=False)
    w_d1c = nc.declare_dram_parameter("w_d1c", [128, 2 * C], BF16, isOutput=False)
    w_d1p = nc.declare_dram_parameter("w_d1p", [SUB, C], BF16, isOutput=False)
    w_d2 = nc.declare_dram_parameter("w_d2", [128, 2 * C], BF16, isOutput=False)
    w_ih = [nc.declare_dram_parameter(f"w_ih{g}", [128, 2 * 3 * C], BF16, isOutput=False)
            for g in range(3)]
    w_hh = [nc.declare_dram_parameter(f"w_hh{g}", [128, 2 * 3 * C], BF16, isOutput=False)
            for g in range(3)]
    w_ow = nc.declare_dram_parameter("w_ow", [128, 2 * SUB], BF16, isOutput=False)
    out = nc.declare_dram_parameter("out", [s_total, SUB, shards * BS], F32, isOutput=True)
    dbg = None
    if debug_d1c:
        dbg = nc.declare_dram_parameter("dbg", [128, nb * 2 * BS], F32, isOutput=True)
        dbg_tmp1 = nc.declare_dram_parameter("dbg_tmp1", [128, 2 * TBQ], BF16, isOutput=True)
        dbg_cv1 = nc.declare_dram_parameter("dbg_cv1", [128, 2 * 202 * BQ], BF16, isOutput=True)
        dbg_cond = nc.declare_dram_parameter("dbg_cond", [128, 2 * nb * BQ], BF16, isOutput=True)

    with tile.TileContext(nc) as tc, ExitStack() as ctx:
        wpool = ctx.enter_context(tc.tile_pool(name="wpool", bufs=1))

        def load(ap, shape, dtype, tag):
            t = wpool.tile(shape, dtype, tag=tag, name=tag)
            nc.sync.dma_start(t[:, :], ap[:, :])
            return t

        sb_featT = load(featT.ap(), [FEAT, 4 * TBQ], BF16, "featT")
        sb_fd1 = load(w_fd1.ap(), [FEAT, C], BF16, "w_fd1")
        sb_c1 = load(w_c1.ap(), [128, 3 * 2 * C], BF16, "w_c1")
        sb_c2 = load(w_c2.ap(), [128, 3 * 2 * C], BF16, "w_c2")
        sb_fd2 = load(w_fd2.ap(), [128, 2 * C], BF16, "w_fd2")
        sb_d1c = load(w_d1c.ap(), [128, 2 * C], BF16, "w_d1c")
        sb_d1p = load(w_d1p.ap(), [SUB, C], BF16, "w_d1p")
        sb_d2 = load(w_d2.ap(), [128, 2 * C], BF16, "w_d2")
        sb_ih = [load(w_ih[g].ap(), [128, 6 * C], BF16, f"w_ih{g}") for g in range(3)]
        sb_hh = [load(w_hh[g].ap(), [128, 6 * C], BF16, f"w_hh{g}") for g in range(3)]
        sb_ow = load(w_ow.ap(), [128, 2 * SUB], BF16, "w_ow")

        # d1c: per shard [128, nb*64] f32; frame f at cols f*64 + m*32 + lane
        d1c = [wpool.tile([128, nb * 2 * BS], F32, tag=f"d1c{sh}", name=f"d1c{sh}") for sh in range(shards)]

        # ---------------- phase 1: cond net + d1c precompute ----------------
        with tc.tile_pool(name="stage", bufs=1) as stage, \
             tc.tile_pool(name="psum1", bufs=4, space="PSUM") as psum1:

            def mm_layer(dst, dst_tb, src, src_tb, w_sb, n_in_blk, cols, taps=None,
                         tap_stride=0):
                """dst[:, m*dst_tb + c] = tanh(sum_{k,kb} W @ src-slice); cols<=dst_tb."""
                for m in range(2):
                    for c0 in range(0, cols, 512):
                        cw = min(512, cols - c0)
                        ps = psum1.tile([128, 512], F32, tag="p1", name="p1")
                        n_acc = (taps or 1) * n_in_blk
                        i = 0
                        for k in range(taps or 1):
                            for kb in range(n_in_blk):
                                wcol = (k * tap_stride if taps else 0) + kb * C + m * 128
                                matmul_args = dict(start=(i == 0), stop=(i == n_acc - 1))
                                nc.tensor.matmul(
                                    ps[:, :cw],
                                    w_sb[:, wcol:wcol + 128],
                                    src[:, kb * src_tb + c0 + (k * BQ if taps else 0):][:, :cw],
                                    **matmul_args,
                                )
                                i += 1
                        nc.scalar.activation(dst[:, m * dst_tb + c0:][:, :cw], ps[:, :cw], AF.Tanh)

            qper = 4 // shards
            for q in range(4):
                sh, hf = q // qper, q % qper
                tb1, tb2, tb3 = 202 * BQ, nb * BQ, nb * BQ
                tmp1 = stage.tile([128, 2 * TBQ], BF16, tag="st1", name="st1")
                # fd1: [20]x[20,128] per m
                for m in range(2):
                    for c0 in range(0, TBQ, 512):
                        cw = min(512, TBQ - c0)
                        ps = psum1.tile([128, 512], F32, tag="p1", name="p1")
                        nc.tensor.matmul(
                            ps[:, :cw], sb_fd1[0:FEAT, m * 128:(m + 1) * 128],
                            sb_featT[0:FEAT, q * TBQ + c0:q * TBQ + c0 + cw],
                            start=True, stop=True)
                        nc.scalar.activation(tmp1[:, m * TBQ + c0:][:, :cw], ps[:, :cw], AF.Tanh)
                cv1 = stage.tile([128, 2 * tb1], BF16, tag="st2", name="st2")
                mm_layer(cv1, tb1, tmp1, TBQ, sb_c1, 2, tb1, taps=3, tap_stride=2 * C)
                cv2 = stage.tile([128, 2 * tb2], BF16, tag="st3", name="st3")
                mm_layer(cv2, tb2, cv1, tb1, sb_c2, 2, tb2, taps=3, tap_stride=2 * C)
                cond = stage.tile([128, 2 * tb3], BF16, tag="st4", name="st4")
                mm_layer(cond, tb3, cv2, tb2, sb_fd2, 2, tb3)
                if dbg is not None and q == 0:
                    nc.sync.dma_start(dbg_tmp1.ap()[:, :], tmp1[:, :])
                    nc.sync.dma_start(dbg_cv1.ap()[:, :], cv1[:, :])
                    nc.sync.dma_start(dbg_cond.ap()[:, :], cond[:, :])
                # d1c
                d1c_r = d1c[sh].rearrange("p (f u) -> p f u", u=2 * BS)
                for m in range(2):
                    for c0 in range(0, tb3, 512):
                        cw = min(512, tb3 - c0)
                        nf = cw // BQ
                        f0 = c0 // BQ
                        ps = psum1.tile([128, 512], F32, tag="p1", name="p1")
                        for kb in range(2):
                            nc.tensor.matmul(
                                ps[:, :cw], sb_d1c[:, kb * C + m * 128:][:, :128],
                                cond[:, kb * tb3 + c0:][:, :cw],
                                start=(kb == 0), stop=(kb == 1))
                        nc.vector.tensor_copy(
                            d1c_r[:, f0:f0 + nf, m * BS + hf * BQ:m * BS + hf * BQ + BQ],
                            ps[:, :cw].rearrange("p (f u) -> p f u", u=BQ))

        if dbg is not None:
            nc.sync.dma_start(dbg.ap()[:, :], d1c[0][:, :])

        # ---------------- phase 2: the scan ----------------
        spool = ctx.enter_context(tc.tile_pool(name="state", bufs=1))
        h_b = [[spool.tile([128, 2 * BS], BF16, tag=f"hb{sh}_{g}", name=f"hb{sh}_{g}") for g in range(3)]
               for sh in range(shards)]
        prev_f = [spool.tile([SUB, BS], F32, tag=f"pf{sh}", name=f"pf{sh}") for sh in range(shards)]
        prev_b = [spool.tile([SUB, BS], BF16, tag=f"pb{sh}", name=f"pb{sh}") for sh in range(shards)]
        for sh in range(shards):
            for g in range(3):
                nc.vector.memset(h_b[sh][g][:, :], 0.0)
            nc.vector.memset(prev_f[sh][:, :], 0.0)
            nc.vector.memset(prev_b[sh][:, :], 0.0)

        gpool = ctx.enter_context(tc.tile_pool(name="gates", bufs=2))
        psG = ctx.enter_context(tc.tile_pool(name="psG", bufs=3, space="PSUM"))
        psM = ctx.enter_context(tc.tile_pool(name="psM", bufs=1, space="PSUM"))

        def emit_step(sh, s):
            f = s // NSUB
            # psG layout [128, 8*BS]: r m0|r m1|z m0|z m1 @ 0..4BS, inn @4..6BS, hn @6..8BS
            # Only the chronologically first matmul into a PSUM bank carries
            # start=True (it clears has_written for the WHOLE bank).
            pg = []
            for g in range(3):
                p = psG.tile([128, 8 * BS], F32, tag=f"psG{sh}", name=f"psG{sh}")
                pg.append(p)
                first = True
                for mp in range(4):
                    for kb in range(2):
                        nc.tensor.matmul(
                            p[:, mp * BS:(mp + 1) * BS],
                            sb_hh[g][:, kb * 3 * C + mp * 128:][:, :128],
                            h_b[sh][g][:, kb * BS:(kb + 1) * BS],
                            start=first, stop=False)
                        first = False
                for m in range(2):
                    for kb in range(2):
                        nc.tensor.matmul(
                            p[:, 6 * BS + m * BS:][:, :BS],
                            sb_hh[g][:, kb * 3 * C + 2 * C + m * 128:][:, :128],
                            h_b[sh][g][:, kb * BS:(kb + 1) * BS],
                            start=False, stop=False)
            # --- d-chain + out share one bank: d1 @0:2BS, d2 @2BS:4BS, out @4BS:5BS
            pm = psM.tile([128, 5 * BS], F32, tag=f"psM{sh}", name=f"psM{sh}")
            for m in range(2):
                nc.tensor.matmul(pm[:, m * BS:(m + 1) * BS],
                                 sb_d1p[0:SUB, m * 128:(m + 1) * 128],
                                 prev_b[sh][0:SUB, :], start=(m == 0), stop=(m == 1))
            nc.vector.tensor_add(pm[:, 0:2 * BS], pm[:, 0:2 * BS],
                                 d1c[sh][:, f * 2 * BS:(f + 1) * 2 * BS])
            t1 = gpool.tile([128, 2 * BS], BF16, tag=f"t1{sh}", name=f"t1{sh}")
            nc.scalar.activation(t1[:, :], pm[:, 0:2 * BS], AF.Tanh)
            for m in range(2):
                for kb in range(2):
                    nc.tensor.matmul(pm[:, 2 * BS + m * BS:][:, :BS],
                                     sb_d2[:, kb * C + m * 128:][:, :128],
                                     t1[:, kb * BS:(kb + 1) * BS],
                                     start=(m == 0 and kb == 0), stop=(m == 1 and kb == 1))
            t2 = gpool.tile([128, 2 * BS], BF16, tag=f"t2{sh}", name=f"t2{sh}")
            nc.scalar.activation(t2[:, :], pm[:, 2 * BS:4 * BS], AF.Tanh)
            # --- GRUs ---
            x = t2
            for g in range(3):
                p = pg[g]
                for mp in range(4):
                    for kb in range(2):
                        nc.tensor.matmul(
                            p[:, mp * BS:(mp + 1) * BS],
                            sb_ih[g][:, kb * 3 * C + mp * 128:][:, :128],
                            x[:, kb * BS:(kb + 1) * BS],
                            start=False, stop=False)
                nmm = 0
                for m in range(2):
                    for kb in range(2):
                        nmm += 1
                        nc.tensor.matmul(
                            p[:, 4 * BS + m * BS:][:, :BS],
                            sb_ih[g][:, kb * 3 * C + 2 * C + m * 128:][:, :128],
                            x[:, kb * BS:(kb + 1) * BS],
                            start=False, stop=(nmm == 4))
                rz = gpool.tile([128, 4 * BS], F32, tag=f"rz{sh}", name=f"rz{sh}")
                nc.scalar.activation(rz[:, :], p[:, 0:4 * BS], AF.Sigmoid)
                r = rz[:, 0:2 * BS]
                z = rz[:, 2 * BS:4 * BS]
                # off-critical: u = z*h and w = 1-z on gpsimd
                u = gpool.tile([128, 2 * BS], F32, tag=f"u{sh}", name=f"u{sh}")
                nc.gpsimd.tensor_mul(u[:, :], z, h_b[sh][g][:, :])
                w = gpool.tile([128, 2 * BS], F32, tag=f"w{sh}", name=f"w{sh}")
                nc.gpsimd.tensor_scalar(w[:, :], z, -1.0, 1.0, ALU.mult, ALU.add)
                rhn = gpool.tile([128, 2 * BS], F32, tag=f"rhn{sh}", name=f"rhn{sh}")
                nc.vector.tensor_mul(rhn[:, :], p[:, 6 * BS:8 * BS], r)
                nc.vector.tensor_add(p[:, 4 * BS:6 * BS], p[:, 4 * BS:6 * BS], rhn[:, :])
                n = gpool.tile([128, 2 * BS], F32, tag=f"n{sh}", name=f"n{sh}")
                nc.scalar.activation(n[:, :], p[:, 4 * BS:6 * BS], AF.Tanh)
                wn = gpool.tile([128, 2 * BS], F32, tag=f"wn{sh}", name=f"wn{sh}")
                nc.vector.tensor_mul(wn[:, :], w[:, :], n[:, :])
                nc.vector.tensor_add(h_b[sh][g][:, :], wn[:, :], u[:, :])
                x = h_b[sh][g]
            # --- output layer into psM @4BS:5BS (rows 0:SUB) ---
            for kb in range(2):
                nc.tensor.matmul(pm[0:SUB, 4 * BS:5 * BS], sb_ow[:, kb * SUB:(kb + 1) * SUB],
                                 h_b[sh][2][:, kb * BS:(kb + 1) * BS],
                                 start=(kb == 0), stop=(kb == 1))
            nc.scalar.activation(prev_f[sh][:, :], pm[0:SUB, 4 * BS:5 * BS], AF.Tanh)
            nc.vector.tensor_copy(prev_b[sh][:, :], prev_f[sh][:, :])
            nc.sync.dma_start(out.ap()[s, :, sh * BS:(sh + 1) * BS], prev_f[sh][:, :])

        for s in range(s_total):
            for sh in range(shards):
                emit_step(sh, s)

    nc.compile()
    return nc


# ---------------- host side ----------------

def _pack_kT(w, nkb):
    """w [out,in] -> lhsT packed [128, nkb*out] bf16 (K-blocks side by side)."""
    wT = np.ascontiguousarray(w.T)  # [in, out]
    blocks = [wT[kb * 128:(kb + 1) * 128] for kb in range(nkb)]
    return np.concatenate(blocks, axis=1).astype(ml_dtypes.bfloat16)


def prep_inputs(inputs, nb=NB, shards=SHARDS):
    ins = {k: np.asarray(v) for k, v in inputs.items()}
    if nb == NB:
        assert int(ins["nb_frames"]) == nb, ins["nb_frames"]
    for bn in ["fd1_b", "c1_b", "c2_b", "fd2_b", "d1_b", "d2_b", "ob",
               "g1_bih", "g1_bhh", "g2_bih", "g2_bhh", "g3_bih", "g3_bhh"]:
        assert np.abs(ins[bn]).max() == 0.0, f"nonzero bias {bn} unsupported"

    weights = {
        "w_fd1": np.ascontiguousarray(ins["fd1_w"].T).astype(ml_dtypes.bfloat16),
        "w_c1": np.concatenate([_pack_kT(ins["c1_w"][:, :, k], 2) for k in range(3)], axis=1),
        "w_c2": np.concatenate([_pack_kT(ins["c2_w"][:, :, k], 2) for k in range(3)], axis=1),
        "w_fd2": _pack_kT(ins["fd2_w"], 2),
        "w_d1c": _pack_kT(ins["d1_w"][:, :C], 2),
        "w_d1p": np.ascontiguousarray(ins["d1_w"][:, C:].T).astype(ml_dtypes.bfloat16),
        "w_d2": _pack_kT(ins["d2_w"], 2),
        "w_ow": _pack_kT(ins["ow"], 2),
    }
    for gi, g in enumerate(["g1", "g2", "g3"]):
        weights[f"w_ih{gi}"] = _pack_kT(ins[g + "_wih"], 2)
        weights[f"w_hh{gi}"] = _pack_kT(ins[g + "_whh"], 2)

    feats = ins["features"]  # [B, T, FEAT] f32
    in_maps = []
    for c in range(NCORES):
        fc = feats[c * 64:(c + 1) * 64]
        qs = []
        for q in range(4):
            blk = fc[q * BQ:(q + 1) * BQ]          # [16, T, FEAT]
            qs.append(blk.transpose(2, 1, 0).reshape(FEAT, T * BQ))
        featT = np.concatenate(qs, axis=1).astype(ml_dtypes.bfloat16)
        im = dict(weights)
        im["featT"] = featT
        in_maps.append(im)
    return in_maps


def assemble(results, nb=NB, shards=SHARDS):
    s_total = nb * NSUB
    rows = []
    for c in range(NCORES):
        arr = np.asarray(results[c]["out"])  # [S, SUB, shards*BS]
        for sh in range(shards):
            blk = arr[:, :, sh * BS:(sh + 1) * BS]       # [S, SUB, BS]
            rows.append(blk.transpose(2, 0, 1).reshape(BS, s_total * SUB))
    return np.concatenate(rows, axis=0).astype(np.float32)


_NC_CACHE = {}


class _CachedRunner:
    """run_bass_via_pjrt with a persistent jitted executable (the stock path
    rebuilds jax.jit per call, re-shipping the program each time)."""

    def __init__(self, nc):
        import jax
        from jax.sharding import Mesh, PartitionSpec
        from jax.experimental.shard_map import shard_map
        from concourse import bass2jax, mybir as _mybir

        bass2jax.install_neuronx_cc_hook()
        self.jax = jax
        partition_name = nc.partition_id_tensor.name if nc.partition_id_tensor else None
        in_names, out_names, out_avals, zero_outs = [], [], [], []
        for alloc in nc.m.functions[0].allocations:
            if not isinstance(alloc, _mybir.MemoryLocationSet):
                continue
            name = alloc.memorylocations[0].name
            if alloc.kind == "ExternalInput":
                if name != partition_name:
                    in_names.append(name)
            elif alloc.kind == "ExternalOutput":
                out_names.append(name)
                shape = tuple(alloc.tensor_shape)
                dtype = _mybir.dt.np(alloc.dtype)
                out_avals.append(jax.core.ShapedArray(shape, dtype))
                zero_outs.append(np.zeros(shape, dtype))
        self.in_names, self.out_names = in_names, out_names
        self.out_avals, self.zero_outs = out_avals, zero_outs
        n_params, n_outs = len(in_names), len(out_avals)
        all_names = list(in_names) + list(out_names)
        if partition_name is not None:
            all_names.append(partition_name)

        def _body(*args):
            operands = list(args)
            if partition_name is not None:
                operands.append(bass2jax.partition_id_tensor())
            outs = bass2jax._bass_exec_p.bind(
                *operands,
                out_avals=tuple(out_avals),
                in_names=tuple(all_names),
                out_names=tuple(out_names),
                lowering_input_output_aliases=(),
                sim_require_finite=True,
                sim_require_nnan=True,
                nc=nc,
            )
            return tuple(outs)

        devices = jax.devices()[:NCORES]
        self.mesh = Mesh(np.asarray(devices), ("core",))
        in_specs = (PartitionSpec("core"),) * (n_params + n_outs)
        out_specs = (PartitionSpec("core"),) * n_outs
        donate = tuple(range(n_params, n_params + n_outs))
        self.fn = jax.jit(
            shard_map(_body, mesh=self.mesh, in_specs=in_specs,
                      out_specs=out_specs, check_rep=False),
            donate_argnums=donate, keep_unused=True)

    def prepare(self, in_maps):
        concat_in = [
            np.concatenate([np.asarray(in_maps[c][n]) for c in range(NCORES)], axis=0)
            for n in self.in_names
        ]
        return concat_in

    def zeros(self):
        return [np.zeros((NCORES * z.shape[0], *z.shape[1:]), z.dtype)
                for z in self.zero_outs]

    def device_zeros(self):
        """Donated output buffers created directly on device (no host transfer)."""
        import jax.numpy as jnp
        from jax.sharding import NamedSharding, PartitionSpec
        sh = NamedSharding(self.mesh, PartitionSpec("core"))
        return [jnp.zeros((NCORES * z.shape[0], *z.shape[1:]), z.dtype, device=sh)
                for z in self.zero_outs]

    def device_inputs(self, concat_in):
        import jax
        from jax.sharding import NamedSharding, PartitionSpec
        sh = NamedSharding(self.mesh, PartitionSpec("core"))
        arrs = [jax.device_put(a, sh) for a in concat_in]
        jax.block_until_ready(arrs)
        return arrs

    def __call__(self, concat_in, concat_zeros):
        out = self.fn(*concat_in, *concat_zeros)
        self.jax.block_until_ready(out)
        return out

    def to_results(self, out_arrs):
        return [
            {n: np.asarray(out_arrs[i]).reshape(NCORES, *self.out_avals[i].shape)[c]
             for i, n in enumerate(self.out_names)}
            for c in range(NCORES)
        ]


def get_runner(nb=NB):
    key = nb
    if key not in _NC_CACHE:
        nc = build_nc(nb=nb)
        _NC_CACHE[key] = _CachedRunner(nc)
    return _NC_CACHE[key]


def run(inputs, nb=NB, trace=False):
    runner = get_runner(nb=nb)
    in_maps = prep_inputs(inputs, nb=nb)
    out_arrs = runner(runner.prepare(in_maps), runner.zeros())
    results = runner.to_results(out_arrs)
    return assemble(results, nb=nb), results


def kernel(**inputs) -> np.ndarray:
    outp, _ = run(inputs, nb=NB, trace=False)
    return outp
